# revision 11
# baseline (speedup 1.0000x reference)
"""Trainium2 Bass kernel for CustomMinkowskiChannelwiseConvolution.

Strategy (graph/data parallel over 8 NeuronCores):
  Host: sort points by flat voxel key, shard output points into 8 contiguous
  ranges (2 halves each so gather indices fit int16), group edges by
  (core, half, tap k, out), and build per-core int16 gather/scatter index
  streams plus a 64-padded feature table window per half.
  Device (per core): for each tap k: dma_gather source feature rows from the
  DRAM window, multiply by the tap's [C] weight row (DVE, broadcast tile),
  dma_scatter_add the weighted rows into the output table (unique out rows
  within a tap, so no RMW races inside a pass; Tile serializes across passes).
  Host: unpermute the gathered per-core outputs back to original row order.
"""

import os
import sys
import types

import numpy as np

import concourse.bacc as bacc
import concourse.tile as tile
from concourse import mybir
from concourse.bass_utils import run_bass_kernel_spmd

LAST_RESULTS = None  # BassKernelResults of the most recent kernel() call


def _wire_ntff_hook():
    """Make run_bass_kernel_spmd(trace=True) work under axon (best effort)."""
    if "antenv.axon_hooks" in sys.modules:
        return
    try:
        import antenv
        from trn_agent_boot.trn_boot import _ntff_profile_via_ctypes

        hook = _ntff_profile_via_ctypes("/opt/axon/libaxon_pjrt.so")
        mod = types.ModuleType("antenv.axon_hooks")
        mod.get_axon_ntff_profile_hook = lambda: hook
        mod.set_axon_ntff_profile_hook = lambda h: None
        sys.modules["antenv.axon_hooks"] = mod
        antenv.axon_hooks = mod
    except Exception:
        pass

# Problem constants (hardcoded per spec).
L = 100
N = 300000
C = 32
KS = 3
R = 1
NTAPS = KS**3  # 27
NCORES = 8

CPAD = 64  # feature rows padded to 64 f32 = 256B (dma_gather granularity)
TAB_ROWS = 32768  # int16-addressable window rows per half
HALO = 6144  # fixed rows of window before each half's own out range
KCEN = NTAPS // 2  # center tap: self edges, handled as bulk DMA


def _round_up(x, m):
    return (x + m - 1) // m * m


def _preprocess(coords, in_idx, out_idx, in_feats, kern, n_cores=NCORES):
    """Sort/shard/group on the host; returns per-core input maps + meta."""
    coords = np.asarray(coords)
    in_idx = np.asarray(in_idx).astype(np.int64)
    out_idx = np.asarray(out_idx).astype(np.int64)
    in_feats = np.asarray(in_feats, dtype=np.float32)
    kern = np.asarray(kern, dtype=np.float32)

    n = in_feats.shape[0]
    c = in_feats.shape[1]
    assert n % (n_cores * 2) == 0, n
    out_per_core = n // n_cores
    half = out_per_core // 2
    out_rows_pad = _round_up(half + 128, 128)  # trailing rows are scratch

    keys = (coords[:, 0].astype(np.int64) * L + coords[:, 1]) * L + coords[:, 2]
    order = np.argsort(keys, kind="stable")
    rank = np.empty(n, np.int64)
    rank[order] = np.arange(n)

    # HALO zero rows before sorted data so each (core, half) window can start
    # at the fixed offset obase-HALO, putting own-out rows at table row HALO.
    feats64 = np.zeros((HALO + n + TAB_ROWS, CPAD), np.float32)
    feats64[HALO : HALO + n, :c] = in_feats[order]

    # per-edge tap id, reference convention
    kc = coords[in_idx] - coords[out_idx] + R  # [E, 3] in [0, KS)
    kk = (kc[:, 0] * KS + kc[:, 1]) * KS + kc[:, 2]

    ii = rank[in_idx]
    oo = rank[out_idx]
    core = oo // out_per_core
    hh = (oo % out_per_core) // half
    grp = (core * 2 + hh) * NTAPS + kk  # 0 .. n_cores*2*27
    ordr = np.lexsort((oo, grp))
    gs = grp[ordr]
    iis = ii[ordr]
    oos = oo[ordr]

    n_grp = n_cores * 2 * NTAPS
    counts = np.bincount(gs, minlength=n_grp).reshape(n_cores, 2, NTAPS)
    e_pad = _round_up(counts.max(axis=(0, 1)), 128)  # [27]
    e_pad[KCEN] = 0  # center tap handled via the bulk contiguous path
    g16 = int(e_pad.sum()) // 16  # idx columns per half

    starts = np.zeros(n_grp + 1, np.int64)
    np.cumsum(np.bincount(gs, minlength=n_grp), out=starts[1:])

    in_maps = []
    wrep = np.zeros((128, NTAPS * CPAD), np.float32)
    for k in range(NTAPS):
        wrep[:, k * CPAD : k * CPAD + c] = kern[k][None, :]

    for cc in range(n_cores):
        gidx = np.zeros((2 * g16, 16), np.int16)
        sidx = np.full((2 * g16, 16), half, np.int16)  # default: scratch row
        for h in range(2):
            g0 = (cc * 2 + h) * NTAPS
            obase = cc * out_per_core + h * half
            # window base (sorted-row coords) fixed at obase-HALO; in feats64
            # the slice starts at index obase (HALO prologue absorbs it).
            b = obase - HALO
            lo = starts[g0]
            hi = starts[g0 + NTAPS]
            assert hi > lo
            assert int(iis[lo:hi].min()) - b >= 0, (cc, h)
            assert int(iis[lo:hi].max()) - b < TAB_ROWS, (cc, h)
            off = h * g16
            for k in range(NTAPS):
                if k == KCEN:
                    continue
                s0, s1 = starts[g0 + k], starts[g0 + k + 1]
                cnt = s1 - s0
                garr = np.zeros(e_pad[k], np.int16)
                sarr = np.full(e_pad[k], half, np.int16)
                garr[:cnt] = (iis[s0:s1] - b).astype(np.int16)
                sarr[:cnt] = (oos[s0:s1] - obase).astype(np.int16)
                ecols = e_pad[k] // 16
                gidx[off : off + ecols] = garr.reshape(ecols, 16)
                sidx[off : off + ecols] = sarr.reshape(ecols, 16)
                off += ecols
        tab0 = feats64[cc * out_per_core : cc * out_per_core + TAB_ROWS]
        tab1 = feats64[cc * out_per_core + half : cc * out_per_core + half + TAB_ROWS]
        # idx layout: index j of a pass -> (partition j%16, col j//16),
        # replicated across the 8 gpsimd core groups -> [128, cols]
        gid128 = np.tile(gidx.T, (8, 1))
        sid128 = np.tile(sidx.T, (8, 1))
        in_maps.append(
            {
                "tab0": np.ascontiguousarray(tab0),
                "tab1": np.ascontiguousarray(tab1),
                "gidx": np.ascontiguousarray(gid128),
                "sidx": np.ascontiguousarray(sid128),
                "wrep": wrep,
            }
        )

    meta = {
        "e_pad": e_pad,
        "g16": g16,
        "rank": rank,
        "out_per_core": out_per_core,
        "half": half,
        "out_rows_pad": out_rows_pad,
        "c": c,
    }
    return in_maps, meta


def _build_program(meta, n_cores=NCORES):
    e_pad = meta["e_pad"]
    g16 = meta["g16"]
    out_rows_pad = meta["out_rows_pad"]
    c = meta["c"]

    nc = bacc.Bacc(
        "TRN2", target_bir_lowering=False, debug=False, num_devices=n_cores
    )
    f32 = mybir.dt.float32
    i16 = mybir.dt.int16
    tabs = [
        nc.dram_tensor(f"tab{h}", [TAB_ROWS, CPAD], f32, kind="ExternalInput")
        for h in range(2)
    ]
    gidx = nc.dram_tensor("gidx", [128, 2 * g16], i16, kind="ExternalInput")
    sidx = nc.dram_tensor("sidx", [128, 2 * g16], i16, kind="ExternalInput")
    wrep = nc.dram_tensor("wrep", [128, NTAPS * CPAD], f32, kind="ExternalInput")
    outb = nc.dram_tensor(
        "outb", [2, out_rows_pad, CPAD], f32, kind="ExternalOutput"
    )

    with tile.TileContext(nc) as tc:
        with (
            tc.tile_pool(name="wpool", bufs=1) as wpool,
            tc.tile_pool(name="ipool", bufs=4) as ipool,
            tc.tile_pool(name="gpool", bufs=2) as gpool,
            tc.tile_pool(name="vpool", bufs=2) as vpool,
        ):
            w_sb = wpool.tile([128, NTAPS * CPAD], f32)
            nc.sync.dma_start(w_sb[:], wrep[:])
            chunk = 1024  # idxs per SWDGE instruction (descriptor-ring cap)
            # center tap: own-out rows sit contiguously at table offset HALO.
            # Bulk load -> weight -> bulk accumulate (no per-edge descriptors).
            rc_rows = 4096
            for h in range(2):
                for r0 in range(0, out_rows_pad, rc_rows):
                    rc = min(rc_rows, out_rows_pad - r0)
                    mc = rc // 128
                    cv = gpool.tile([128, mc, CPAD], f32, tag="cv")
                    nc.sync.dma_start(
                        cv[:],
                        tabs[h][HALO + r0 : HALO + r0 + rc, :].rearrange(
                            "(p j) c -> p j c", p=128
                        ),
                    )
                    cw = vpool.tile([128, mc, CPAD], f32, tag="cw")
                    nc.vector.tensor_tensor(
                        out=cw[:],
                        in0=cv[:],
                        in1=w_sb[:, KCEN * CPAD : (KCEN + 1) * CPAD]
                        .unsqueeze(1)
                        .to_broadcast([128, mc, CPAD]),
                        op=mybir.AluOpType.mult,
                    )
                    nc.gpsimd.dma_start(
                        outb[h, r0 : r0 + rc, :].rearrange(
                            "(p j) c -> p j c", p=128
                        ),
                        cw[:],
                        accum_op=mybir.AluOpType.add,
                    )
            off = 0
            for h in range(2):
                for k in range(NTAPS):
                    if k == KCEN:
                        continue
                    ep = int(e_pad[k])
                    for e0 in range(0, ep, chunk):
                        ec = min(chunk, ep - e0)
                        m = ec // 128
                        cols = ec // 16
                        c0 = off + e0 // 16
                        gt = ipool.tile([128, cols], i16, tag="gt")
                        nc.sync.dma_start(gt[:], gidx[:, c0 : c0 + cols])
                        st = ipool.tile([128, cols], i16, tag="st")
                        nc.sync.dma_start(st[:], sidx[:, c0 : c0 + cols])
                        gv = gpool.tile([128, m, CPAD], f32, tag="gv")
                        nc.gpsimd.dma_gather(
                            gv[:], tabs[h][:, :], gt[:], ec, ec, CPAD,
                            single_packet=True,
                        )
                        wv = vpool.tile([128, m, c], f32, tag="wv")
                        nc.vector.tensor_tensor(
                            out=wv[:],
                            in0=gv[:, :, 0:c],
                            in1=w_sb[:, k * CPAD : k * CPAD + c]
                            .unsqueeze(1)
                            .to_broadcast([128, m, c]),
                            op=mybir.AluOpType.mult,
                        )
                        nc.gpsimd.dma_scatter_add(
                            outb[h, :, 0:c],
                            wv[:],
                            st[:],
                            ec,
                            ec,
                            c,
                            elem_step=CPAD,
                            single_packet=True,
                        )
                    off += ep // 16
    nc.compile()
    return nc


def kernel(**inputs):
    coords = inputs["coords"]
    in_idx = inputs["in_idx"]
    out_idx = inputs["out_idx"]
    in_feats = inputs["in_feats"]
    kern = inputs["kernel"]

    in_maps, meta = _preprocess(coords, in_idx, out_idx, in_feats, kern)
    nc = _build_program(meta)
    trace = os.environ.get("MINK_TRACE", "0") == "1"
    if trace:
        _wire_ntff_hook()
    res = run_bass_kernel_spmd(
        nc, in_maps, core_ids=list(range(NCORES)), trace=trace
    )
    global LAST_RESULTS
    LAST_RESULTS = res

    n = np.asarray(in_feats).shape[0]
    c = meta["c"]
    half = meta["half"]
    out_per_core = meta["out_per_core"]
    out_s = np.empty((n, c), np.float32)
    for cc in range(NCORES):
        ob = res.results[cc]["outb"]
        for h in range(2):
            base = cc * out_per_core + h * half
            out_s[base : base + half] = ob[h, :half, :c]
    return out_s[meta["rank"]]


if __name__ == "__main__":
    rng = np.random.default_rng(0)
    print("kernel module ok")


# revision 18
# speedup vs baseline: 1.2462x; 1.2462x over previous
"""Trainium2 Bass kernel for CustomMinkowskiChannelwiseConvolution.

Strategy (graph/data parallel over 8 NeuronCores):
  Host: sort points by flat voxel key, shard output points into 8 contiguous
  ranges (2 halves each so gather indices fit int16), group edges by
  (core, half, tap k, out), and build per-core int16 gather/scatter index
  streams plus a 64-padded feature table window per half.
  Device (per core): for each tap k: dma_gather source feature rows from the
  DRAM window, multiply by the tap's [C] weight row (DVE, broadcast tile),
  dma_scatter_add the weighted rows into the output table (unique out rows
  within a tap, so no RMW races inside a pass; Tile serializes across passes).
  Host: unpermute the gathered per-core outputs back to original row order.
"""

import os
import sys
import types

import numpy as np

import concourse.bacc as bacc
import concourse.tile as tile
from concourse import mybir
from concourse.bass_utils import run_bass_kernel_spmd

LAST_RESULTS = None  # BassKernelResults of the most recent kernel() call


def _wire_ntff_hook():
    """Make run_bass_kernel_spmd(trace=True) work under axon (best effort)."""
    if "antenv.axon_hooks" in sys.modules:
        return
    try:
        import antenv
        from trn_agent_boot.trn_boot import _ntff_profile_via_ctypes

        hook = _ntff_profile_via_ctypes("/opt/axon/libaxon_pjrt.so")
        mod = types.ModuleType("antenv.axon_hooks")
        mod.get_axon_ntff_profile_hook = lambda: hook
        mod.set_axon_ntff_profile_hook = lambda h: None
        sys.modules["antenv.axon_hooks"] = mod
        antenv.axon_hooks = mod
    except Exception:
        pass

# Problem constants (hardcoded per spec).
L = 100
N = 300000
C = 32
KS = 3
R = 1
NTAPS = KS**3  # 27
NCORES = 8

CPAD = 64  # feature rows padded to 64 f32 = 256B (dma_gather granularity)
TAB_ROWS = 32768  # int16-addressable window rows per half
HALO = 6144  # fixed rows of window before each half's own out range
KCEN = NTAPS // 2  # center tap: self edges, handled as bulk DMA
KZM = KCEN - 1  # tap (0,0,-1): source row = out row - 1 where mask set
KZP = KCEN + 1  # tap (0,0,+1): source row = out row + 1 where mask set


def _round_up(x, m):
    return (x + m - 1) // m * m


def _preprocess(coords, in_idx, out_idx, in_feats, kern, n_cores=NCORES):
    """Sort/shard/group on the host; returns per-core input maps + meta."""
    coords = np.asarray(coords)
    in_idx = np.asarray(in_idx).astype(np.int64)
    out_idx = np.asarray(out_idx).astype(np.int64)
    in_feats = np.asarray(in_feats, dtype=np.float32)
    kern = np.asarray(kern, dtype=np.float32)

    n = in_feats.shape[0]
    c = in_feats.shape[1]
    assert n % (n_cores * 2) == 0, n
    out_per_core = n // n_cores
    half = out_per_core // 2
    out_rows_pad = _round_up(half + 128, 128)  # trailing rows are scratch

    keys = (coords[:, 0].astype(np.int64) * L + coords[:, 1]) * L + coords[:, 2]
    order = np.argsort(keys, kind="stable")
    rank = np.empty(n, np.int64)
    rank[order] = np.arange(n)

    # HALO zero rows before sorted data so each (core, half) window can start
    # at the fixed offset obase-HALO, putting own-out rows at table row HALO.
    feats64 = np.zeros((HALO + n + TAB_ROWS, CPAD), np.float32)
    feats64[HALO : HALO + n, :c] = in_feats[order]

    # per-edge tap id, reference convention
    kc = coords[in_idx] - coords[out_idx] + R  # [E, 3] in [0, KS)
    kk = (kc[:, 0] * KS + kc[:, 1]) * KS + kc[:, 2]

    ii = rank[in_idx]
    oo = rank[out_idx]
    core = oo // out_per_core
    hh = (oo % out_per_core) // half
    grp = (core * 2 + hh) * NTAPS + kk  # 0 .. n_cores*2*27
    ordr = np.lexsort((oo, grp))
    gs = grp[ordr]
    iis = ii[ordr]
    oos = oo[ordr]

    n_grp = n_cores * 2 * NTAPS
    counts = np.bincount(gs, minlength=n_grp).reshape(n_cores, 2, NTAPS)
    e_pad = _round_up(counts.max(axis=(0, 1)), 128)  # [27]
    # center + z-neighbor taps go through the bulk contiguous (shift) path
    e_pad[KCEN] = 0
    e_pad[KZM] = 0
    e_pad[KZP] = 0
    g16 = int(e_pad.sum()) // 16  # idx columns per half

    # z+-1 existence masks over sorted rows (tap exists <=> adjacent sorted
    # key differs by exactly 1 and z stays in bounds)
    keys_s = keys[order]
    z_s = coords[order][:, 2].astype(np.int64)
    mzp = np.zeros(n + 2 * TAB_ROWS, np.float32)
    mzm = np.zeros(n + 2 * TAB_ROWS, np.float32)
    mzp[: n - 1] = ((keys_s[1:] == keys_s[:-1] + 1) & (z_s[:-1] < L - 1)).astype(
        np.float32
    )
    mzm[1:n] = ((keys_s[:-1] == keys_s[1:] - 1) & (z_s[1:] > 0)).astype(np.float32)

    starts = np.zeros(n_grp + 1, np.int64)
    np.cumsum(np.bincount(gs, minlength=n_grp), out=starts[1:])

    in_maps = []
    wrep = np.zeros((128, NTAPS * CPAD), np.float32)
    for k in range(NTAPS):
        wrep[:, k * CPAD : k * CPAD + c] = kern[k][None, :]

    for cc in range(n_cores):
        gidx = np.zeros((2 * g16, 16), np.int16)
        sidx = np.full((2 * g16, 16), half, np.int16)  # default: scratch row
        for h in range(2):
            g0 = (cc * 2 + h) * NTAPS
            obase = cc * out_per_core + h * half
            # window base (sorted-row coords) fixed at obase-HALO; in feats64
            # the slice starts at index obase (HALO prologue absorbs it).
            b = obase - HALO
            lo = starts[g0]
            hi = starts[g0 + NTAPS]
            assert hi > lo
            assert int(iis[lo:hi].min()) - b >= 0, (cc, h)
            assert int(iis[lo:hi].max()) - b < TAB_ROWS, (cc, h)
            off = h * g16
            for k in range(NTAPS):
                if k in (KCEN, KZM, KZP):
                    continue
                s0, s1 = starts[g0 + k], starts[g0 + k + 1]
                cnt = s1 - s0
                garr = np.zeros(e_pad[k], np.int16)
                sarr = np.full(e_pad[k], half, np.int16)
                garr[:cnt] = (iis[s0:s1] - b).astype(np.int16)
                sarr[:cnt] = (oos[s0:s1] - obase).astype(np.int16)
                ecols = e_pad[k] // 16
                gidx[off : off + ecols] = garr.reshape(ecols, 16)
                sidx[off : off + ecols] = sarr.reshape(ecols, 16)
                off += ecols
        tab0 = feats64[cc * out_per_core : cc * out_per_core + TAB_ROWS]
        tab1 = feats64[cc * out_per_core + half : cc * out_per_core + half + TAB_ROWS]
        # per-row (mask x weight) tables for the z+-1 shift path
        zw = np.zeros((2, 2, out_rows_pad, CPAD), np.float32)
        for h in range(2):
            obase = cc * out_per_core + h * half
            zw[h, 0, :, :c] = (
                mzm[obase : obase + out_rows_pad, None] * kern[KZM][None, :]
            )
            zw[h, 1, :, :c] = (
                mzp[obase : obase + out_rows_pad, None] * kern[KZP][None, :]
            )
        # idx layout: index j of a pass -> (partition j%16, col j//16),
        # replicated across the 8 gpsimd core groups -> [128, cols]
        gid128 = np.tile(gidx.T, (8, 1))
        sid128 = np.tile(sidx.T, (8, 1))
        in_maps.append(
            {
                "tab0": np.ascontiguousarray(tab0),
                "tab1": np.ascontiguousarray(tab1),
                "gidx": np.ascontiguousarray(gid128),
                "sidx": np.ascontiguousarray(sid128),
                "wrep": wrep,
                "zw": zw,
            }
        )

    meta = {
        "e_pad": e_pad,
        "g16": g16,
        "rank": rank,
        "out_per_core": out_per_core,
        "half": half,
        "out_rows_pad": out_rows_pad,
        "c": c,
    }
    return in_maps, meta


def _build_program(meta, n_cores=NCORES):
    e_pad = meta["e_pad"]
    g16 = meta["g16"]
    out_rows_pad = meta["out_rows_pad"]
    c = meta["c"]

    nc = bacc.Bacc(
        "TRN2", target_bir_lowering=False, debug=False, num_devices=n_cores
    )
    f32 = mybir.dt.float32
    i16 = mybir.dt.int16
    tabs = [
        nc.dram_tensor(f"tab{h}", [TAB_ROWS, CPAD], f32, kind="ExternalInput")
        for h in range(2)
    ]
    gidx = nc.dram_tensor("gidx", [128, 2 * g16], i16, kind="ExternalInput")
    sidx = nc.dram_tensor("sidx", [128, 2 * g16], i16, kind="ExternalInput")
    wrep = nc.dram_tensor("wrep", [128, NTAPS * CPAD], f32, kind="ExternalInput")
    zw = nc.dram_tensor(
        "zw", [2, 2, out_rows_pad, CPAD], f32, kind="ExternalInput"
    )
    outb = nc.dram_tensor(
        "outb", [2, out_rows_pad, CPAD], f32, kind="ExternalOutput"
    )

    with tile.TileContext(nc) as tc:
        with (
            tc.tile_pool(name="wpool", bufs=1) as wpool,
            tc.tile_pool(name="ipool", bufs=4) as ipool,
            tc.tile_pool(name="gpool", bufs=2) as gpool,
            tc.tile_pool(name="vpool", bufs=2) as vpool,
        ):
            w_sb = wpool.tile([128, NTAPS * CPAD], f32)
            nc.sync.dma_start(w_sb[:], wrep[:])
            chunk = 2048  # idxs per SWDGE instruction (descriptor-ring cap)
            # center + z+-1 taps: own-out rows sit contiguously at table
            # offset HALO; z neighbors are the adjacent sorted rows where the
            # host mask is set. Bulk load -> weight/mask -> bulk accumulate
            # (no per-edge descriptors).
            rc_rows = 4096
            for h in range(2):
                for r0 in range(0, out_rows_pad, rc_rows):
                    rc = min(rc_rows, out_rows_pad - r0)
                    mc = rc // 128

                    def ldtab(shift, tag):
                        t = gpool.tile([128, mc, CPAD], f32, tag=tag)
                        nc.sync.dma_start(
                            t[:],
                            tabs[h][
                                HALO + r0 + shift : HALO + r0 + rc + shift, :
                            ].rearrange("(p j) c -> p j c", p=128),
                        )
                        return t

                    cv = ldtab(0, "cv")
                    cvm = ldtab(-1, "cvm")
                    cvp = ldtab(1, "cvp")
                    zwt = []
                    for t in range(2):
                        zt = gpool.tile([128, mc, CPAD], f32, tag=f"zw{t}")
                        nc.sync.dma_start(
                            zt[:],
                            zw[h, t, r0 : r0 + rc, :].rearrange(
                                "(p j) c -> p j c", p=128
                            ),
                        )
                        zwt.append(zt)
                    cw = vpool.tile([128, mc, CPAD], f32, tag="cw")
                    nc.vector.tensor_tensor(
                        out=cw[:],
                        in0=cv[:],
                        in1=w_sb[:, KCEN * CPAD : (KCEN + 1) * CPAD]
                        .unsqueeze(1)
                        .to_broadcast([128, mc, CPAD]),
                        op=mybir.AluOpType.mult,
                    )
                    for t, src in ((0, cvm), (1, cvp)):
                        tt = vpool.tile([128, mc, CPAD], f32, tag="tt")
                        nc.vector.tensor_tensor(
                            out=tt[:],
                            in0=src[:],
                            in1=zwt[t][:],
                            op=mybir.AluOpType.mult,
                        )
                        nc.vector.tensor_tensor(
                            out=cw[:],
                            in0=cw[:],
                            in1=tt[:],
                            op=mybir.AluOpType.add,
                        )
                    nc.gpsimd.dma_start(
                        outb[h, r0 : r0 + rc, :].rearrange(
                            "(p j) c -> p j c", p=128
                        ),
                        cw[:],
                        accum_op=mybir.AluOpType.add,
                    )
            off = 0
            for h in range(2):
                for k in range(NTAPS):
                    if k == KCEN:
                        continue
                    ep = int(e_pad[k])
                    for e0 in range(0, ep, chunk):
                        ec = min(chunk, ep - e0)
                        m = ec // 128
                        cols = ec // 16
                        c0 = off + e0 // 16
                        gt = ipool.tile([128, cols], i16, tag="gt")
                        nc.sync.dma_start(gt[:], gidx[:, c0 : c0 + cols])
                        st = ipool.tile([128, cols], i16, tag="st")
                        nc.sync.dma_start(st[:], sidx[:, c0 : c0 + cols])
                        gv = gpool.tile([128, m, CPAD], f32, tag="gv")
                        nc.gpsimd.dma_gather(
                            gv[:], tabs[h][:, :], gt[:], ec, ec, CPAD,
                            single_packet=False,
                        )
                        wv = vpool.tile([128, m, c], f32, tag="wv")
                        nc.vector.tensor_tensor(
                            out=wv[:],
                            in0=gv[:, :, 0:c],
                            in1=w_sb[:, k * CPAD : k * CPAD + c]
                            .unsqueeze(1)
                            .to_broadcast([128, m, c]),
                            op=mybir.AluOpType.mult,
                        )
                        nc.gpsimd.dma_scatter_add(
                            outb[h, :, 0:c],
                            wv[:],
                            st[:],
                            ec,
                            ec,
                            c,
                            elem_step=CPAD,
                            single_packet=False,
                        )
                    off += ep // 16
    nc.compile()
    return nc


def kernel(**inputs):
    coords = inputs["coords"]
    in_idx = inputs["in_idx"]
    out_idx = inputs["out_idx"]
    in_feats = inputs["in_feats"]
    kern = inputs["kernel"]

    in_maps, meta = _preprocess(coords, in_idx, out_idx, in_feats, kern)
    nc = _build_program(meta)
    trace = os.environ.get("MINK_TRACE", "0") == "1"
    if trace:
        _wire_ntff_hook()
    res = run_bass_kernel_spmd(
        nc, in_maps, core_ids=list(range(NCORES)), trace=trace
    )
    global LAST_RESULTS
    LAST_RESULTS = res

    n = np.asarray(in_feats).shape[0]
    c = meta["c"]
    half = meta["half"]
    out_per_core = meta["out_per_core"]
    out_s = np.empty((n, c), np.float32)
    for cc in range(NCORES):
        ob = res.results[cc]["outb"]
        for h in range(2):
            base = cc * out_per_core + h * half
            out_s[base : base + half] = ob[h, :half, :c]
    return out_s[meta["rank"]]


if __name__ == "__main__":
    rng = np.random.default_rng(0)
    print("kernel module ok")
